# revision 12
# baseline (speedup 1.0000x reference)
"""PatchCore anomaly score kernel for 8 trn2 NeuronCores.

score = sqrt(max_n min_m ||patches[n] - memory_bank[m]||^2)

Device (per core, memory_bank sharded 4096 rows/core):
  r_c[n] = max_m (2*patches[n].bank[m] - (m_sq[m] - C))     [MAX-only ops]
Host:
  min_d2[n] = p_sq[n] + C - max_c r_c[n];  score = sqrt(max_n min_d2)
"""

import sys

import numpy as np

try:
    import concourse.bass as bass
except ImportError:
    sys.path.insert(0, "/opt/trn_rl_repo")
    import concourse.bass as bass

import concourse.bacc as bacc
import concourse.tile as tile
from concourse import mybir
from concourse.bass_utils import run_bass_kernel_spmd

import ml_dtypes

N = 8192          # patches
M_TOTAL = 32768   # memory bank rows
D = 512           # feature dim
N_CORES = 8
M = M_TOTAL // N_CORES   # 4096 bank rows per core

KP = 4            # k-chunks of 128 over D
NT = N // 512     # 16 n-tiles of 512 patches
MT = M // 128     # 32 m-tiles of 128 bank rows


def _build_nc():
    # Bacc (not Bass): its compile() pass splits multi-sem waits into
    # event semaphores — TRN2 allows only 1 embedded wait per instruction.
    nc = bacc.Bacc(None, target_bir_lowering=False)
    f32 = mybir.dt.float32
    bf16 = mybir.dt.bfloat16

    at_d = nc.dram_tensor("at", [D, N], bf16, kind="ExternalInput")
    bt_d = nc.dram_tensor("bt", [D, M], bf16, kind="ExternalInput")
    msq_d = nc.dram_tensor("msq", [128, MT], f32, kind="ExternalInput")
    id_d = nc.dram_tensor("ident", [128, 128], bf16, kind="ExternalInput")
    # out[p, blk] = r[blk*128 + p]; host transposes
    out_d = nc.dram_tensor("out", [128, NT * 4], f32, kind="ExternalOutput")

    with tile.TileContext(nc) as tc:
        with (
            tc.tile_pool(name="at", bufs=1) as at_pool,
            tc.tile_pool(name="bt", bufs=1) as bt_pool,
            tc.tile_pool(name="msq", bufs=1) as msq_pool,
            tc.tile_pool(name="rmax", bufs=2) as rmax_pool,
            tc.tile_pool(name="evac", bufs=4) as evac_pool,
            tc.tile_pool(name="res", bufs=1) as res_pool,
            tc.tile_pool(name="psum", bufs=6, space="PSUM") as psum_pool,
            tc.tile_pool(name="psumt", bufs=2, space="PSUM") as psumt_pool,
        ):
            msq_t = msq_pool.tile([128, MT], f32)
            nc.gpsimd.dma_start(msq_t[:], msq_d[:])
            id_t = msq_pool.tile([128, 128], bf16, name="id_t")
            nc.gpsimd.dma_start(id_t[:], id_d[:])
            res_t = res_pool.tile([128, NT * 4], f32)

            # bank first (whole bank needed for n-tile 0), in k/col chunks
            bt_t = [bt_pool.tile([128, M], bf16, name=f"bt{k}") for k in range(KP)]
            for k in range(KP):
                for j in range(4):
                    nc.gpsimd.dma_start(
                        bt_t[k][:, bass.ts(j, M // 4)],
                        bt_d[bass.ts(k, 128), bass.ts(j, M // 4)],
                    )
            # patches, in column chunks matching n-tile consumption order
            at_t = [at_pool.tile([128, N], bf16, name=f"at{k}") for k in range(KP)]
            for j in range(8):
                for k in range(KP):
                    nc.gpsimd.dma_start(
                        at_t[k][:, bass.ts(j, N // 8)],
                        at_d[bass.ts(k, 128), bass.ts(j, N // 8)],
                    )

            def reduce_ntile(n, rmax):
                # partition-axis max of rmax [128,512] via PE transpose
                # of each 128-col chunk + DVE free-axis max reduce.
                for c in range(4):
                    pst = psumt_pool.tile([128, 128], bf16, name="pst")
                    nc.tensor.transpose(
                        pst[:], rmax[:, bass.ts(c, 128)], id_t[:]
                    )
                    col = n * 4 + c
                    nc.vector.tensor_reduce(
                        res_t[:, col : col + 1], pst[:],
                        mybir.AxisListType.X, mybir.AluOpType.max,
                    )

            prev = None
            for n in range(NT):
                rmax = rmax_pool.tile([128, 512], bf16)
                for m in range(MT):
                    ps = psum_pool.tile([128, 512], f32)
                    for k in range(KP):
                        nc.tensor.matmul(
                            ps[:],
                            bt_t[k][:, bass.ts(m, 128)],
                            at_t[k][:, bass.ts(n, 512)],
                            start=(k == 0),
                            stop=(k == KP - 1),
                        )
                    if m == 0:
                        nc.scalar.activation(
                            rmax[:], ps[:], mybir.ActivationFunctionType.Identity,
                            bias=msq_t[:, m : m + 1], scale=2.0,
                        )
                    else:
                        ev = evac_pool.tile([128, 512], bf16)
                        nc.scalar.activation(
                            ev[:], ps[:], mybir.ActivationFunctionType.Identity,
                            bias=msq_t[:, m : m + 1], scale=2.0,
                        )
                        nc.vector.tensor_max(rmax[:], rmax[:], ev[:])
                    if m == 4 and prev is not None:
                        reduce_ntile(n - 1, prev)  # keep PE busy before stall
                prev = rmax
            reduce_ntile(NT - 1, prev)
            nc.gpsimd.dma_start(out_d[:], res_t[:])

    nc.finalize()
    return nc


_NC = None


def kernel(patches: np.ndarray, memory_bank: np.ndarray) -> np.ndarray:
    global _NC
    if _NC is None:
        _NC = _build_nc()
    nc = _NC

    p64 = patches.astype(np.float64)
    b64 = memory_bank.astype(np.float64)
    p_sq = np.sum(p64 * p64, axis=1)          # [N]
    m_sq = np.sum(b64 * b64, axis=1)          # [M_TOTAL]
    C = float(np.mean(m_sq))

    at_np = np.ascontiguousarray(patches.T).astype(ml_dtypes.bfloat16)
    id_np = np.eye(128, dtype=ml_dtypes.bfloat16)
    in_maps = []
    for c in range(N_CORES):
        bank_c = memory_bank[c * M : (c + 1) * M]
        bt_np = np.ascontiguousarray(bank_c.T).astype(ml_dtypes.bfloat16)
        msq_c = -(m_sq[c * M : (c + 1) * M] - C)
        msq_np = np.ascontiguousarray(
            msq_c.reshape(MT, 128).T
        ).astype(np.float32)
        in_maps.append({"at": at_np, "bt": bt_np, "msq": msq_np, "ident": id_np})

    br = run_bass_kernel_spmd(nc, in_maps, list(range(N_CORES)))
    r = np.max(
        np.stack(
            [np.asarray(br.results[c]["out"], np.float64).T.reshape(N)
             for c in range(N_CORES)]
        ),
        axis=0,
    )
    min_d2 = np.maximum(p_sq + C - r, 0.0)
    score = np.sqrt(np.max(min_d2))
    return np.asarray(score, dtype=np.float32)
